# revision 14
# baseline (speedup 1.0000x reference)
"""LlamaSkipMLP Trainium2 kernel.

Strategy: data-parallel over the token dim across 8 NeuronCores (no
collectives).  Each core computes out_c = silu(x_c@Wg'.T) * (x_c@Wu'.T) @ Wd'.T
for its 1024-token slice, where Wg'/Wu'/Wd' are the active-neuron
gather of the weights (done host-side; for active_idx = arange(k) it
is a plain slice).

Device kernel (per core, Tile framework):
  phase 1: g/u GEMMs contract hidden dim H (on PE partitions), fused
           SiLU*up on ACT+DVE, h stored [k_part, t_free] in fp16.
  phase 2: down GEMM contracts the active-neuron dim k; h tiles serve
           as the stationary operand, W_down^T tiles as the moving
           operand, so the output lands as [t_part, h_free] and stores
           contiguously.

All matmuls run in fp16 (PSUM accumulates fp32).  Host pre-lays-out
weights/activations so every DMA is contiguous and no on-device
transposes are needed.
"""

import numpy as np

# Problem shapes (hardcoded per spec).
T, H, K = 8192, 4096, 3302
NCORES = 8
KP = 3328                 # K padded to a multiple of 128
NK0 = KP // 128           # 26 k-tiles
NH0 = H // 128            # 32 h-tiles (contraction, phase 1)
TC = T // NCORES          # 1024 tokens per core

_CACHE = {}


def build_nc(kp=KP, h=H, tct=TC, enable_asserts=False):
    """Build + compile the per-core Bass program (SPMD: same on all cores)."""
    from contextlib import ExitStack

    import concourse.mybir as mybir
    import concourse.tile as tile
    from concourse import bacc

    fp16 = mybir.dt.float16
    fp32 = mybir.dt.float32
    Sigmoid = mybir.ActivationFunctionType.Sigmoid

    nk0 = kp // 128
    nh0 = h // 128
    ntf = tct // 512          # moving t-tiles, phase 1
    nt1 = tct // 128          # stationary t-tiles, phase 2
    nhf = h // 512            # moving h-tiles, phase 2

    nc = bacc.Bacc(
        "TRN2", target_bir_lowering=False, debug=False,
        enable_asserts=enable_asserts,
    )
    xt = nc.dram_tensor("xt", [128, nh0 * tct], fp16, kind="ExternalInput").ap()
    wg = nc.dram_tensor("wg", [nk0, 128, nh0 * 128], fp16, kind="ExternalInput").ap()
    wu = nc.dram_tensor("wu", [nk0, 128, nh0 * 128], fp16, kind="ExternalInput").ap()
    wd = nc.dram_tensor("wd", [nk0, 128, h], fp16, kind="ExternalInput").ap()
    # fp16 output halves the final out-DMA wave (host upcasts to fp32).
    out = nc.dram_tensor("out", [tct, h], fp16, kind="ExternalOutput").ap()

    with tile.TileContext(nc) as tc, ExitStack() as ctx:
        xt_pool = ctx.enter_context(tc.tile_pool(name="xtp", bufs=1))
        w_pool = ctx.enter_context(tc.tile_pool(name="wp", bufs=3))
        wd_pool = ctx.enter_context(tc.tile_pool(name="wdp", bufs=8))
        h_pool = ctx.enter_context(tc.tile_pool(name="hp", bufs=1))
        tmp_pool = ctx.enter_context(tc.tile_pool(name="tmpp", bufs=4))
        out_pool = ctx.enter_context(tc.tile_pool(name="outp", bufs=8))

        xt_sb = xt_pool.tile([128, nh0 * tct], fp16, name="xt_sb")
        h_sb = h_pool.tile([128, nk0 * tct], fp16, name="h_sb")

        # PE warm-up: the HAM clock gate keeps the PE at 1.2GHz until it has
        # seen ~3.4us of sustained matmul activity.  Real matmuls can't start
        # until the first weight/x DMAs land (~12us in), so without this the
        # first ~25 real matmuls run at half clock.  Burn 16 dummy matmuls on
        # zeroed scratch tiles right after the start barrier; they overlap
        # the initial DMAs and hand the real stream a warm (2.4GHz) PE.
        warm_w = tmp_pool.tile([128, 128], fp16, name="warm_w", tag="warm_w")
        warm_m = tmp_pool.tile([128, 512], fp16, name="warm_m", tag="warm_m")
        nc.vector.memset(warm_w[:, :], 0.0)
        nc.vector.memset(warm_m[:, :], 0.0)
        with tc.tile_pool(name="psw", space="PSUM", bufs=1) as psw:
            warm_p = psw.tile([128, 512], fp32, name="warm_p")
            for _ in range(8):
                nc.tensor.matmul(warm_p[:, :], warm_w[:, :], warm_m[:, :],
                                 start=True, stop=True)

        # Prologue: the first matmul (k0=0, h0=0) needs only wg[0]'s first
        # h0-blocks and x^T's first columns.  Issue those as small leading
        # DMAs so MM#1 isn't gated on the full 1MB wg[0] slab; the rest
        # follows in coarser chunks, interleaved so each lands before the
        # PE consumes it (gate MMs eat one h0-block per ~430ns).
        # The Sync engine's first ~9us are occupied by a framework barrier +
        # a serial 4.2us TENSOR_LOAD, so the leading chunks go out on the
        # ACT engine's DGE queue (idle at program start) — transfers begin
        # ~6us earlier and the PE never idles long enough to re-throttle.
        wg_t0 = w_pool.tile([128, nh0 * 128], fp16, name="wg_t", tag="wg")
        wu_t0 = w_pool.tile([128, nh0 * 128], fp16, name="wu_t", tag="wu")
        nc.scalar.dma_start(wg_t0[:, 0:512], wg[0, :, 0:512])
        nc.scalar.dma_start(xt_sb[:, 0:2 * tct], xt[:, 0:2 * tct])
        nc.scalar.dma_start(wg_t0[:, 512:2048], wg[0, :, 512:2048])
        nc.scalar.dma_start(xt_sb[:, 2 * tct:8 * tct], xt[:, 2 * tct:8 * tct])
        nc.sync.dma_start(wg_t0[:, 2048:nh0 * 128], wg[0, :, 2048:nh0 * 128])
        nc.sync.dma_start(wu_t0[:, :], wu[0])
        # remaining x^T in 2-h0-block chunks (512KB each)
        for i in range(8, nh0, 2):
            nc.sync.dma_start(xt_sb[:, i * tct:(i + 2) * tct],
                              xt[:, i * tct:(i + 2) * tct])

        # ---- phase 1: g = x@Wg^T, u = x@Wu^T, h = silu(g)*u ----
        with tc.tile_pool(name="ps1", space="PSUM", bufs=2) as ps1:
            for k0 in range(nk0):
                if k0 == 0:
                    wg_t, wu_t = wg_t0, wu_t0
                else:
                    # Halved slab DMAs: the k0's first matmuls gate on half
                    # a slab, so each k0 can start while its tail streams.
                    wg_t = w_pool.tile([128, nh0 * 128], fp16, name="wg_t", tag="wg")
                    nc.sync.dma_start(wg_t[:, 0:2048], wg[k0, :, 0:2048])
                    nc.sync.dma_start(wg_t[:, 2048:nh0 * 128],
                                      wg[k0, :, 2048:nh0 * 128])
                    wu_t = w_pool.tile([128, nh0 * 128], fp16, name="wu_t", tag="wu")
                    nc.sync.dma_start(wu_t[:, 0:2048], wu[k0, :, 0:2048])
                    nc.sync.dma_start(wu_t[:, 2048:nh0 * 128],
                                      wu[k0, :, 2048:nh0 * 128])
                pg = [ps1.tile([128, 512], fp32, name=f"pg{i}", tag=f"pg{i}")
                      for i in range(ntf)]
                pu = [ps1.tile([128, 512], fp32, name=f"pu{i}", tag=f"pu{i}")
                      for i in range(ntf)]
                for h0 in range(nh0):
                    for i in range(ntf):
                        nc.tensor.matmul(
                            pg[i][:, :], wg_t[:, h0 * 128:(h0 + 1) * 128],
                            xt_sb[:, h0 * tct + i * 512:h0 * tct + (i + 1) * 512],
                            start=(h0 == 0), stop=(h0 == nh0 - 1),
                        )
                for h0 in range(nh0):
                    for i in range(ntf):
                        nc.tensor.matmul(
                            pu[i][:, :], wu_t[:, h0 * 128:(h0 + 1) * 128],
                            xt_sb[:, h0 * tct + i * 512:h0 * tct + (i + 1) * 512],
                            start=(h0 == 0), stop=(h0 == nh0 - 1),
                        )
                for i in range(ntf):
                    sg = tmp_pool.tile([128, 512], fp32, name="sg", tag="sg")
                    nc.scalar.activation(sg[:, :], pg[i][:, :], Sigmoid)
                    sl = tmp_pool.tile([128, 512], fp32, name="sl", tag="sl")
                    nc.vector.tensor_mul(sl[:, :], sg[:, :], pg[i][:, :])
                    nc.vector.tensor_mul(
                        h_sb[:, k0 * tct + i * 512:k0 * tct + (i + 1) * 512],
                        sl[:, :], pu[i][:, :])

        # ---- phase 2: out = h @ Wd^T (contract k) ----
        Copy = mybir.ActivationFunctionType.Copy
        with tc.tile_pool(name="ps2", space="PSUM", bufs=1) as ps2:
            # hf=0 is split into two t1-halves: its first matmuls otherwise
            # wait ~1.6us for the last phase-1 silu drains to free the PSUM
            # banks that po[4..7] land on.  The first half only needs 4
            # banks (free immediately); the second half re-DMAs wd (cheap).
            passes = [(0, range(0, nt1 // 2)), (0, range(nt1 // 2, nt1))] + [
                (hf, range(nt1)) for hf in range(1, nhf)
            ]
            for hf, t1s in passes:
                t1s = list(t1s)
                po = {t1: ps2.tile([128, 512], fp32, name=f"po{t1}", tag=f"po{t1}")
                      for t1 in t1s}
                for k0 in range(nk0):
                    wd_t = wd_pool.tile([128, 512], fp16, name="wd_t", tag="wd")
                    nc.sync.dma_start(wd_t[:, :], wd[k0, :, hf * 512:(hf + 1) * 512])
                    for t1 in t1s:
                        nc.tensor.matmul(
                            po[t1][:, :],
                            h_sb[:, k0 * tct + t1 * 128:k0 * tct + (t1 + 1) * 128],
                            wd_t[:, :],
                            start=(k0 == 0), stop=(k0 == nk0 - 1),
                        )
                # Drains alternate DVE / ACT so the two engines empty the
                # PSUM banks in parallel; the ~0.6us DMA-issue cost likewise
                # alternates between the two HWDGE queues (Sync, ACT).  The
                # very last tile is split into two halves drained+issued on
                # both engine pairs in parallel to shorten the kernel tail.
                last = (hf == nhf - 1)
                for j, t1 in enumerate(t1s):
                    ot = out_pool.tile([128, 512], fp16, name="ot", tag="ot")
                    orow = out[t1 * 128:(t1 + 1) * 128, hf * 512:(hf + 1) * 512]
                    if last and t1 == nt1 - 1:
                        nc.vector.tensor_copy(ot[:, 0:256], po[t1][:, 0:256])
                        nc.scalar.activation(ot[:, 256:512], po[t1][:, 256:512],
                                             Copy)
                        nc.sync.dma_start(orow[:, 0:256], ot[:, 0:256])
                        nc.scalar.dma_start(orow[:, 256:512], ot[:, 256:512])
                        continue
                    if j % 2 == 0:
                        nc.vector.tensor_copy(ot[:, :], po[t1][:, :])
                        nc.sync.dma_start(orow, ot[:, :])
                    else:
                        nc.scalar.activation(ot[:, :], po[t1][:, :], Copy)
                        nc.scalar.dma_start(orow, ot[:, :])

    nc.compile()
    return nc


def prep_weights(W_gate, W_up, W_down, active_idx, kp=KP, h=H):
    idx = np.asarray(active_idx)
    k = idx.shape[0]
    nk0 = kp // 128
    nh0 = h // 128

    def lay_gu(W):
        a = np.zeros((kp, h), np.float16)
        a[:k] = W[idx].astype(np.float16)
        # [k0, p, h0*128 + k_in] = a[k0*128+k_in, h0*128+p]
        return np.ascontiguousarray(
            a.reshape(nk0, 128, nh0, 128).transpose(0, 3, 2, 1)
        ).reshape(nk0, 128, nh0 * 128)

    wd_a = np.zeros((kp, h), np.float16)
    wd_a[:k] = W_down[:, idx].T.astype(np.float16)
    wd_prep = np.ascontiguousarray(wd_a.reshape(nk0, 128, h))
    return lay_gu(W_gate), lay_gu(W_up), wd_prep


def prep_x_core(xc, h=H, tct=TC):
    nh0 = h // 128
    xt_c = np.ascontiguousarray(
        xc.astype(np.float16).T.reshape(nh0, 128, tct).transpose(1, 0, 2))
    return xt_c.reshape(128, nh0 * tct)


def run(inputs, trace=False, **kw):
    from concourse.bass_utils import run_bass_kernel_spmd

    if "nc" not in _CACHE:
        _CACHE["nc"] = build_nc()
    nc = _CACHE["nc"]

    wg_prep, wu_prep, wd_prep = prep_weights(
        inputs["W_gate"], inputs["W_up"], inputs["W_down"], inputs["active_idx"])
    x = inputs["x"]
    in_maps = [
        {"xt": prep_x_core(x[c * TC:(c + 1) * TC]),
         "wg": wg_prep, "wu": wu_prep, "wd": wd_prep}
        for c in range(NCORES)
    ]
    res = run_bass_kernel_spmd(nc, in_maps, core_ids=list(range(NCORES)),
                               trace=trace, **kw)
    out = np.concatenate(
        [res.results[c]["out"].astype(np.float32) for c in range(NCORES)], axis=0)
    return out, res


def kernel(**inputs):
    out, _ = run(inputs, trace=False)
    return out



# revision 16
# speedup vs baseline: 1.0039x; 1.0039x over previous
"""LlamaSkipMLP Trainium2 kernel.

Strategy: data-parallel over the token dim across 8 NeuronCores (no
collectives).  Each core computes out_c = silu(x_c@Wg'.T) * (x_c@Wu'.T) @ Wd'.T
for its 1024-token slice, where Wg'/Wu'/Wd' are the active-neuron
gather of the weights (done host-side; for active_idx = arange(k) it
is a plain slice).

Device kernel (per core, Tile framework):
  phase 1: g/u GEMMs contract hidden dim H (on PE partitions), fused
           SiLU*up on ACT+DVE, h stored [k_part, t_free] in fp16.
  phase 2: down GEMM contracts the active-neuron dim k; h tiles serve
           as the stationary operand, W_down^T tiles as the moving
           operand, so the output lands as [t_part, h_free] and stores
           contiguously.

All matmuls run in fp16 (PSUM accumulates fp32).  Host pre-lays-out
weights/activations so every DMA is contiguous and no on-device
transposes are needed.
"""

import numpy as np

# Problem shapes (hardcoded per spec).
T, H, K = 8192, 4096, 3302
NCORES = 8
KP = 3328                 # K padded to a multiple of 128
NK0 = KP // 128           # 26 k-tiles
NH0 = H // 128            # 32 h-tiles (contraction, phase 1)
TC = T // NCORES          # 1024 tokens per core

_CACHE = {}


def build_nc(kp=KP, h=H, tct=TC, enable_asserts=False):
    """Build + compile the per-core Bass program (SPMD: same on all cores)."""
    from contextlib import ExitStack

    import concourse.mybir as mybir
    import concourse.tile as tile
    from concourse import bacc

    fp16 = mybir.dt.float16
    fp32 = mybir.dt.float32
    Sigmoid = mybir.ActivationFunctionType.Sigmoid

    nk0 = kp // 128
    nh0 = h // 128
    ntf = tct // 512          # moving t-tiles, phase 1
    nt1 = tct // 128          # stationary t-tiles, phase 2
    nhf = h // 512            # moving h-tiles, phase 2

    nc = bacc.Bacc(
        "TRN2", target_bir_lowering=False, debug=False,
        enable_asserts=enable_asserts,
    )
    xt = nc.dram_tensor("xt", [128, nh0 * tct], fp16, kind="ExternalInput").ap()
    wg = nc.dram_tensor("wg", [nk0, 128, nh0 * 128], fp16, kind="ExternalInput").ap()
    wu = nc.dram_tensor("wu", [nk0, 128, nh0 * 128], fp16, kind="ExternalInput").ap()
    wd = nc.dram_tensor("wd", [nk0, 128, h], fp16, kind="ExternalInput").ap()
    # fp16 output halves the final out-DMA wave (host upcasts to fp32).
    out = nc.dram_tensor("out", [tct, h], fp16, kind="ExternalOutput").ap()

    with tile.TileContext(nc) as tc, ExitStack() as ctx:
        xt_pool = ctx.enter_context(tc.tile_pool(name="xtp", bufs=1))
        w_pool = ctx.enter_context(tc.tile_pool(name="wp", bufs=3))
        wd_pool = ctx.enter_context(tc.tile_pool(name="wdp", bufs=8))
        h_pool = ctx.enter_context(tc.tile_pool(name="hp", bufs=1))
        tmp_pool = ctx.enter_context(tc.tile_pool(name="tmpp", bufs=4))
        out_pool = ctx.enter_context(tc.tile_pool(name="outp", bufs=8))

        xt_sb = xt_pool.tile([128, nh0 * tct], fp16, name="xt_sb")
        h_sb = h_pool.tile([128, nk0 * tct], fp16, name="h_sb")

        # PE warm-up: the HAM clock gate keeps the PE at 1.2GHz until it has
        # seen ~3.4us of sustained matmul activity.  Real matmuls can't start
        # until the first weight/x DMAs land (~12us in), so without this the
        # first ~25 real matmuls run at half clock.  Burn 16 dummy matmuls on
        # zeroed scratch tiles right after the start barrier; they overlap
        # the initial DMAs and hand the real stream a warm (2.4GHz) PE.
        warm_w = tmp_pool.tile([128, 128], fp16, name="warm_w", tag="warm_w")
        warm_m = tmp_pool.tile([128, 512], fp16, name="warm_m", tag="warm_m")
        nc.vector.memset(warm_w[:, :], 0.0)
        nc.vector.memset(warm_m[:, :], 0.0)
        with tc.tile_pool(name="psw", space="PSUM", bufs=1) as psw:
            warm_p = psw.tile([128, 512], fp32, name="warm_p")
            for _ in range(8):
                nc.tensor.matmul(warm_p[:, :], warm_w[:, :], warm_m[:, :],
                                 start=True, stop=True)

        # Prologue: the first matmul (k0=0, h0=0) needs only wg[0]'s first
        # h0-blocks and x^T's first columns.  Issue those as small leading
        # DMAs so MM#1 isn't gated on the full 1MB wg[0] slab; the rest
        # follows in coarser chunks, interleaved so each lands before the
        # PE consumes it (gate MMs eat one h0-block per ~430ns).
        # All prologue DMAs go on the single Sync DGE queue: both queues
        # share the same 16 DMA engines, so a second queue only lets the
        # bulk chunks steal bandwidth from the critical first chunks; a
        # single FIFO makes issue order = delivery priority.  Chunks are
        # ordered by first-use and sized so MM#1 gates on ~320KB.
        wg_t0 = w_pool.tile([128, nh0 * 128], fp16, name="wg_t", tag="wg")
        wu_t0 = w_pool.tile([128, nh0 * 128], fp16, name="wu_t", tag="wu")
        nc.sync.dma_start(wg_t0[:, 0:256], wg[0, :, 0:256])
        nc.sync.dma_start(xt_sb[:, 0:tct], xt[:, 0:tct])
        nc.sync.dma_start(wg_t0[:, 256:1024], wg[0, :, 256:1024])
        nc.sync.dma_start(xt_sb[:, tct:4 * tct], xt[:, tct:4 * tct])
        nc.sync.dma_start(wg_t0[:, 1024:2048], wg[0, :, 1024:2048])
        nc.sync.dma_start(xt_sb[:, 4 * tct:8 * tct], xt[:, 4 * tct:8 * tct])
        nc.sync.dma_start(wg_t0[:, 2048:nh0 * 128], wg[0, :, 2048:nh0 * 128])
        nc.sync.dma_start(wu_t0[:, :], wu[0])
        # remaining x^T in 2-h0-block chunks (512KB each)
        for i in range(8, nh0, 2):
            nc.sync.dma_start(xt_sb[:, i * tct:(i + 2) * tct],
                              xt[:, i * tct:(i + 2) * tct])

        # ---- phase 1: g = x@Wg^T, u = x@Wu^T, h = silu(g)*u ----
        with tc.tile_pool(name="ps1", space="PSUM", bufs=2) as ps1:
            for k0 in range(nk0):
                if k0 == 0:
                    wg_t, wu_t = wg_t0, wu_t0
                else:
                    # Halved slab DMAs: the k0's first matmuls gate on half
                    # a slab, so each k0 can start while its tail streams.
                    wg_t = w_pool.tile([128, nh0 * 128], fp16, name="wg_t", tag="wg")
                    nc.sync.dma_start(wg_t[:, 0:2048], wg[k0, :, 0:2048])
                    nc.sync.dma_start(wg_t[:, 2048:nh0 * 128],
                                      wg[k0, :, 2048:nh0 * 128])
                    wu_t = w_pool.tile([128, nh0 * 128], fp16, name="wu_t", tag="wu")
                    nc.sync.dma_start(wu_t[:, 0:2048], wu[k0, :, 0:2048])
                    nc.sync.dma_start(wu_t[:, 2048:nh0 * 128],
                                      wu[k0, :, 2048:nh0 * 128])
                pg = [ps1.tile([128, 512], fp32, name=f"pg{i}", tag=f"pg{i}")
                      for i in range(ntf)]
                pu = [ps1.tile([128, 512], fp32, name=f"pu{i}", tag=f"pu{i}")
                      for i in range(ntf)]
                for h0 in range(nh0):
                    for i in range(ntf):
                        nc.tensor.matmul(
                            pg[i][:, :], wg_t[:, h0 * 128:(h0 + 1) * 128],
                            xt_sb[:, h0 * tct + i * 512:h0 * tct + (i + 1) * 512],
                            start=(h0 == 0), stop=(h0 == nh0 - 1),
                        )
                for h0 in range(nh0):
                    for i in range(ntf):
                        nc.tensor.matmul(
                            pu[i][:, :], wu_t[:, h0 * 128:(h0 + 1) * 128],
                            xt_sb[:, h0 * tct + i * 512:h0 * tct + (i + 1) * 512],
                            start=(h0 == 0), stop=(h0 == nh0 - 1),
                        )
                for i in range(ntf):
                    sg = tmp_pool.tile([128, 512], fp32, name="sg", tag="sg")
                    nc.scalar.activation(sg[:, :], pg[i][:, :], Sigmoid)
                    sl = tmp_pool.tile([128, 512], fp32, name="sl", tag="sl")
                    nc.vector.tensor_mul(sl[:, :], sg[:, :], pg[i][:, :])
                    nc.vector.tensor_mul(
                        h_sb[:, k0 * tct + i * 512:k0 * tct + (i + 1) * 512],
                        sl[:, :], pu[i][:, :])

        # ---- phase 2: out = h @ Wd^T (contract k) ----
        Copy = mybir.ActivationFunctionType.Copy
        with tc.tile_pool(name="ps2", space="PSUM", bufs=1) as ps2:
            # hf=0 is split into two t1-halves: its first matmuls otherwise
            # wait ~1.6us for the last phase-1 silu drains to free the PSUM
            # banks that po[4..7] land on.  The first half only needs 4
            # banks (free immediately); the second half re-DMAs wd (cheap).
            # The last hf is likewise split: its 8 PSUM drains (~5.4us over
            # 2 engines) can't hide inside its final 1.7us of matmuls, so a
            # 4-tile final pass halves the post-matmul drain spill.
            passes = (
                [(0, range(0, nt1 // 2)), (0, range(nt1 // 2, nt1))]
                + [(hf, range(nt1)) for hf in range(1, nhf - 1)]
                + [(nhf - 1, range(0, nt1 // 2)), (nhf - 1, range(nt1 // 2, nt1))]
            )
            for hf, t1s in passes:
                t1s = list(t1s)
                po = {t1: ps2.tile([128, 512], fp32, name=f"po{t1}", tag=f"po{t1}")
                      for t1 in t1s}
                for k0 in range(nk0):
                    wd_t = wd_pool.tile([128, 512], fp16, name="wd_t", tag="wd")
                    nc.sync.dma_start(wd_t[:, :], wd[k0, :, hf * 512:(hf + 1) * 512])
                    for t1 in t1s:
                        nc.tensor.matmul(
                            po[t1][:, :],
                            h_sb[:, k0 * tct + t1 * 128:k0 * tct + (t1 + 1) * 128],
                            wd_t[:, :],
                            start=(k0 == 0), stop=(k0 == nk0 - 1),
                        )
                # Drains alternate DVE / ACT so the two engines empty the
                # PSUM banks in parallel; the ~0.6us DMA-issue cost likewise
                # alternates between the two HWDGE queues (Sync, ACT).  The
                # very last tile is split into two halves drained+issued on
                # both engine pairs in parallel to shorten the kernel tail.
                last = (hf == nhf - 1)
                for j, t1 in enumerate(t1s):
                    ot = out_pool.tile([128, 512], fp16, name="ot", tag="ot")
                    orow = out[t1 * 128:(t1 + 1) * 128, hf * 512:(hf + 1) * 512]
                    if last and t1 == nt1 - 1:
                        nc.vector.tensor_copy(ot[:, 0:256], po[t1][:, 0:256])
                        nc.scalar.activation(ot[:, 256:512], po[t1][:, 256:512],
                                             Copy)
                        nc.sync.dma_start(orow[:, 0:256], ot[:, 0:256])
                        nc.scalar.dma_start(orow[:, 256:512], ot[:, 256:512])
                        continue
                    if j % 2 == 0:
                        nc.vector.tensor_copy(ot[:, :], po[t1][:, :])
                        nc.sync.dma_start(orow, ot[:, :])
                    else:
                        nc.scalar.activation(ot[:, :], po[t1][:, :], Copy)
                        nc.scalar.dma_start(orow, ot[:, :])

    nc.compile()
    return nc


def prep_weights(W_gate, W_up, W_down, active_idx, kp=KP, h=H):
    idx = np.asarray(active_idx)
    k = idx.shape[0]
    nk0 = kp // 128
    nh0 = h // 128

    def lay_gu(W):
        a = np.zeros((kp, h), np.float16)
        a[:k] = W[idx].astype(np.float16)
        # [k0, p, h0*128 + k_in] = a[k0*128+k_in, h0*128+p]
        return np.ascontiguousarray(
            a.reshape(nk0, 128, nh0, 128).transpose(0, 3, 2, 1)
        ).reshape(nk0, 128, nh0 * 128)

    wd_a = np.zeros((kp, h), np.float16)
    wd_a[:k] = W_down[:, idx].T.astype(np.float16)
    wd_prep = np.ascontiguousarray(wd_a.reshape(nk0, 128, h))
    return lay_gu(W_gate), lay_gu(W_up), wd_prep


def prep_x_core(xc, h=H, tct=TC):
    nh0 = h // 128
    xt_c = np.ascontiguousarray(
        xc.astype(np.float16).T.reshape(nh0, 128, tct).transpose(1, 0, 2))
    return xt_c.reshape(128, nh0 * tct)


def run(inputs, trace=False, **kw):
    from concourse.bass_utils import run_bass_kernel_spmd

    if "nc" not in _CACHE:
        _CACHE["nc"] = build_nc()
    nc = _CACHE["nc"]

    wg_prep, wu_prep, wd_prep = prep_weights(
        inputs["W_gate"], inputs["W_up"], inputs["W_down"], inputs["active_idx"])
    x = inputs["x"]
    in_maps = [
        {"xt": prep_x_core(x[c * TC:(c + 1) * TC]),
         "wg": wg_prep, "wu": wu_prep, "wd": wd_prep}
        for c in range(NCORES)
    ]
    res = run_bass_kernel_spmd(nc, in_maps, core_ids=list(range(NCORES)),
                               trace=trace, **kw)
    out = np.concatenate(
        [res.results[c]["out"].astype(np.float32) for c in range(NCORES)], axis=0)
    return out, res


def kernel(**inputs):
    out, _ = run(inputs, trace=False)
    return out

